# revision 1
# baseline (speedup 1.0000x reference)
"""Trainium2 Bass kernel for nn_ConvTran (conv stem + eRPE transformer + GAP).

Sharding: pure data parallel. B=16 split as 2 samples per core across 8 cores.
All parameters replicated; per-core outputs concatenated on host.

v2: bf16 matmuls (fp32 matmul costs 4 cyc/row vs 1 for bf16), FD=1024
activation tiles, ln/exp-based layernorm rstd (avoids sqrt table swaps),
relu on DVE, batched LN stats.
"""

import numpy as np
import ml_dtypes

# ---- problem constants (hardcoded; kernel.py must be self-contained) ----
B, S, C_IN, E, H, DFF = 16, 1024, 4, 128, 8, 512
C1 = E * 4          # 512
DH = E // H         # 16
EPS = 1e-5
SCALE = float(E) ** -0.5
N_CORES = 8
NB = B // N_CORES   # samples per core = 2
NG = 2              # head groups of 4
SC = S // 128       # 8 s-chunks
JC = S // 128       # 8 j-chunks
F32 = np.float32
BF16 = ml_dtypes.bfloat16


class _Pack:
    """Column-packed [128, N] constant store."""

    def __init__(self, dtype):
        self.dtype = dtype
        self.cols = []
        self.index = {}
        self.n = 0

    def add(self, name, arr2d):
        a = np.zeros((128, arr2d.shape[1]), self.dtype)
        a[:arr2d.shape[0]] = arr2d.astype(self.dtype)
        self.index[name] = (self.n, arr2d.shape[1])
        self.cols.append(a)
        self.n += arr2d.shape[1]

    def finalize(self):
        return np.ascontiguousarray(np.concatenate(self.cols, axis=1))


def _host_prep(inp):
    f = lambda a: np.asarray(a, dtype=F32)
    pb = _Pack(BF16)   # matmul operands
    pf = _Pack(F32)    # activation scale/bias scalars

    # conv1 weights, 4 copies at 32-row offsets for row-tiled matmuls:
    # w1c4[32*cc + t, j] = conv1_w[cc*128 + j, 0, 0, t]
    w1 = f(inp["conv1_w"])[:, 0, 0, :]                 # [C1, 8]
    w1c4 = np.zeros((128, 128), F32)
    for cc in range(4):
        w1c4[32 * cc:32 * cc + 8, :] = w1[cc * 128:(cc + 1) * 128, :].T
    pb.add("w1c4", w1c4)
    sA = f(inp["bn1_g"]) / np.sqrt(f(inp["bn1_v"]) + EPS)
    pf.add("scaleA", sA.reshape(4, 128).T)
    pf.add("biasA", ((f(inp["conv1_b"]) - f(inp["bn1_m"])) * sA
                     + f(inp["bn1_b"])).reshape(4, 128).T)

    # conv2: [128(c1), 16(k=r*4+cc), 128(e)]
    w2 = f(inp["conv2_w"])[:, :, :, 0]                 # [E, C1, 4]
    w2cT = np.zeros((128, 16, 128), F32)
    for r in range(4):
        for cc in range(4):
            w2cT[:, r * 4 + cc, :] = w2[:, cc * 128:(cc + 1) * 128, r].T
    pb.add("w2cT", w2cT.reshape(128, 16 * 128))
    sB = f(inp["bn2_g"]) / np.sqrt(f(inp["bn2_v"]) + EPS)
    pf.add("scaleB", sB[:, None])
    pf.add("biasB", ((f(inp["conv2_b"]) - f(inp["bn2_m"])) * sB
                     + f(inp["bn2_b"]))[:, None])

    # tAPE fixed positional encoding, transposed [E, S]
    pos = np.arange(S, dtype=np.float64)[:, None]
    div = np.exp(np.arange(0, E, 2, dtype=np.float64) * (-np.log(10000.0) / E))
    ang = pos * div * (E / S)
    pe = np.zeros((S, E), np.float64)
    pe[:, 0::2] = np.sin(ang)
    pe[:, 1::2] = np.cos(ang)
    pb.add("peT", pe.T)

    # q/k weights, padded head layout [128, g*128 + 32c + dh]
    def pad_qk(w):
        w = f(w)
        wt = np.zeros((128, NG * 128), F32)
        for g in range(NG):
            for c in range(4):
                h = 4 * g + c
                wt[:, g * 128 + 32 * c:g * 128 + 32 * c + DH] = \
                    w[h * DH:(h + 1) * DH, :].T
        return wt
    pb.add("wqT", pad_qk(inp["wq"]))
    pb.add("wkT", pad_qk(inp["wk"]))
    pb.add("wvT", f(inp["wv"]).T)

    pb.add("ffw1T", f(inp["ff_w1"]).T)
    pf.add("ffb1", f(inp["ff_b1"]).reshape(4, 128).T)
    pb.add("ffw2T", f(inp["ff_w2"]).T.reshape(4, 128, 128)
           .transpose(1, 0, 2).reshape(128, 512))
    pf.add("ffb2", f(inp["ff_b2"])[:, None])

    m = np.arange(128)
    pf.add("bcast4", (m[None, :] // 32 == np.arange(4)[:, None]).astype(F32))
    pf.add("eps", np.full((128, 1), EPS, F32))
    pb.add("identB", np.eye(128, dtype=F32))
    pb.add("onesB", np.ones((128, 1), F32))

    lnG = np.stack([f(inp["ln_attn_g"]), f(inp["ln1_g"]), f(inp["ln2_g"])])
    lnB = np.stack([f(inp["ln_attn_b"]), f(inp["ln1_b"]), f(inp["ln2_b"])])
    ln_identity = bool(np.allclose(lnG, 1.0) and np.allclose(lnB, 0.0))
    pf.add("lnG", np.broadcast_to(lnG.reshape(1, 3 * 128), (128, 384)).copy())
    pf.add("lnB", np.broadcast_to(lnB.reshape(1, 3 * 128), (128, 384)).copy())

    d = {"cpk16": pb.finalize(), "cpk32": pf.finalize()}

    # rel_bias diagonal store (bf16): T[jj, h, c] = rel[127 + c - jj, h]
    rel = f(inp["rel_bias"])                          # [2047, 8]
    jj = np.arange(128)[:, None]
    cidx = np.arange(1920)[None, :]
    ts = rel[127 + cidx - jj, :]                      # [128, 1920, 8]
    d["tstore"] = np.ascontiguousarray(
        ts.transpose(0, 2, 1).astype(BF16))           # [128, 8, 1920]
    return d, pb.index, pf.index, ln_identity


def _build_bass(i16, i32, n16, n32, ln_identity):
    import concourse.bass as bass
    import concourse.bacc as bacc
    import concourse.tile as tile
    import concourse.mybir as mybir

    dt = mybir.dt
    AF = mybir.ActivationFunctionType
    ALU = mybir.AluOpType

    nc = bacc.Bacc("TRN2")

    xin = nc.dram_tensor("rhs8x", [NB, 128, 4 * S], dt.bfloat16,
                         kind="ExternalInput")
    c16_dr = nc.dram_tensor("cpk16", [128, n16], dt.bfloat16,
                            kind="ExternalInput")
    c32_dr = nc.dram_tensor("cpk32", [128, n32], dt.float32,
                            kind="ExternalInput")
    ts_dr = nc.dram_tensor("tstore", [128, H, 1920], dt.bfloat16,
                           kind="ExternalInput")
    yout = nc.dram_tensor("y", [NB, E], dt.float32, kind="ExternalOutput")

    with tile.TileContext(nc) as tc:
        import contextlib
        ctx = contextlib.ExitStack()
        with ctx:
            consts = ctx.enter_context(tc.tile_pool(name="consts", bufs=1))
            c16 = consts.tile([128, n16], dt.bfloat16, tag="cpk16")
            nc.sync.dma_start(out=c16, in_=c16_dr[:])
            c32 = consts.tile([128, n32], dt.float32, tag="cpk32")
            nc.sync.dma_start(out=c32, in_=c32_dr[:])
            ts_sb = consts.tile([128, H, 1920], dt.bfloat16, tag="tstore")
            nc.sync.dma_start(out=ts_sb, in_=ts_dr[:])

            def C16(name, rows=128):
                o, w = i16[name]
                return c16[0:rows, o:o + w]

            def C32(name, rows=128):
                o, w = i32[name]
                return c32[0:rows, o:o + w]

            w1c4_sb = C16("w1c4")
            scaleA_sb, biasA_sb = C32("scaleA"), C32("biasA")
            w2cT_sb = C16("w2cT").rearrange("p (k e) -> p k e", k=16)
            scaleB_sb, biasB_sb = C32("scaleB"), C32("biasB")
            peT_sb = C16("peT")
            wqT_sb = C16("wqT").rearrange("p (g e) -> p g e", g=NG)
            wkT_sb = C16("wkT").rearrange("p (g e) -> p g e", g=NG)
            wvT_sb = C16("wvT")
            ffw1T_sb = C16("ffw1T")
            ffb1_sb = C32("ffb1")
            ffw2T_sb = C16("ffw2T").rearrange("p (k e) -> p k e", k=4)
            ffb2_sb = C32("ffb2")
            bcast4_sb = C32("bcast4", rows=4)
            eps_sb = C32("eps")
            identB = C16("identB")
            onesB = C16("onesB")
            lnG_sb = C32("lnG").rearrange("p (k e) -> p k e", k=3)
            lnB_sb = C32("lnB").rearrange("p (k e) -> p k e", k=3)

            pers = ctx.enter_context(tc.tile_pool(name="pers", bufs=1))
            xpT = [pers.tile([128, S], dt.bfloat16, tag=f"xpT{b}",
                             name=f"xpT{b}") for b in range(NB)]
            xsrc = [pers.tile([128, SC, 128], dt.bfloat16, tag=f"xsrc{b}",
                              name=f"xsrc{b}") for b in range(NB)]

            # =========== PHASE C: conv stem (both samples) ===========
            with tc.tile_pool(name="h1", bufs=1) as h1p, \
                 tc.tile_pool(name="cps", bufs=2, space="PSUM") as cps, \
                 tc.tile_pool(name="cmisc", bufs=2) as cmisc:
                for b in range(NB):
                    rhs8 = cmisc.tile([128, 4 * S], dt.bfloat16, tag="rhs8",
                                      name="rhs8")
                    # split the load so conv1 r=0 starts on the first chunk
                    for rq in range(4):
                        nc.sync.dma_start(
                            out=rhs8[:, rq * S:(rq + 1) * S],
                            in_=xin[b][:, rq * S:(rq + 1) * S])
                    h1 = [[h1p.tile([128, S], dt.bfloat16,
                                    tag=f"h1_{cc}_{r}", name=f"h1_{cc}_{r}")
                           for r in range(4)] for cc in range(4)]
                    # conv1 + BN1 + gelu, row-tiled over cc
                    for r in range(4):
                        for cc in range(4):
                            ps = cps.tile([128, 2, 512], dt.float32,
                                          tag="c1ps", name="c1ps", bufs=3)
                            for sh in range(2):
                                n = r * 2 + sh
                                nc.tensor.matmul(
                                    ps[:, sh, :],
                                    lhsT=w1c4_sb[32 * cc:32 * cc + 8, :],
                                    rhs=rhs8[32 * cc:32 * cc + 8,
                                             n * 512:(n + 1) * 512],
                                    start=True, stop=True,
                                    tile_position=(32 * cc, 0))
                            nc.scalar.activation(
                                h1[cc][r], ps.rearrange("p a b -> p (a b)"),
                                AF.Gelu, bias=biasA_sb[:, cc:cc + 1],
                                scale=scaleA_sb[:, cc:cc + 1])
                    # conv2 + BN2 + gelu
                    xsT = cmisc.tile([128, S], dt.bfloat16, tag="xsT",
                                     name="xsT")
                    ps2 = cps.tile([128, 2, 512], dt.float32, tag="c2ps",
                                   name="c2ps", bufs=1)
                    for sh in range(2):
                        for k in range(16):
                            r, cc = k // 4, k % 4
                            nc.tensor.matmul(
                                ps2[:, sh, :], lhsT=w2cT_sb[:, k, :],
                                rhs=h1[cc][r][:, sh * 512:(sh + 1) * 512],
                                start=(k == 0), stop=(k == 15))
                    nc.scalar.activation(
                        xsT, ps2.rearrange("p a b -> p (a b)"), AF.Gelu,
                        bias=biasB_sb, scale=scaleB_sb)
                    nc.vector.tensor_tensor(xpT[b], xsT, peT_sb, op=ALU.add)
                    nc.sync.dma_start_transpose(out=xsrc[b], in_=xsT)

            # =========== attention + tail pools ===========
            scp = ctx.enter_context(
                tc.tile_pool(name="scp", bufs=2, space="PSUM"))
            accp = ctx.enter_context(
                tc.tile_pool(name="accp", bufs=1, space="PSUM"))
            msp = ctx.enter_context(
                tc.tile_pool(name="msp", bufs=2, space="PSUM"))
            qkv = ctx.enter_context(tc.tile_pool(name="qkv", bufs=2))
            att_p = ctx.enter_context(tc.tile_pool(name="attp", bufs=2))
            sm = ctx.enter_context(tc.tile_pool(name="sm", bufs=2))

            oatt = {}
            QKV = {}
            for b in range(NB):
                # ---------- QKV ----------
                qT, kT = [], []
                for g in range(NG):
                    qt = qkv.tile([128, S], dt.bfloat16, tag=f"qt{g}",
                                  name=f"qt{g}")
                    kt = qkv.tile([128, S], dt.bfloat16, tag=f"kt{g}",
                                  name=f"kt{g}")
                    for dst, w in ((qt, wqT_sb), (kt, wkT_sb)):
                        for sh in range(2):
                            ps = msp.tile([128, 512], dt.float32, tag="ms",
                                          name="msqk")
                            nc.tensor.matmul(ps, lhsT=w[:, g, :],
                                             rhs=xpT[b][:, sh * 512:(sh + 1) * 512],
                                             start=True, stop=True)
                            nc.vector.tensor_copy(
                                out=dst[:, sh * 512:(sh + 1) * 512], in_=ps)
                    qT.append(qt)
                    kT.append(kt)
                v_sb = qkv.tile([128, SC, 128], dt.bfloat16, tag="v", name="v")
                for half in range(2):
                    ps = msp.tile([128, 4, 128], dt.float32, tag="ms",
                                  name="msv")
                    for q4 in range(4):
                        sc = half * 4 + q4
                        nc.tensor.matmul(
                            ps[:, q4, :],
                            lhsT=xpT[b][:, sc * 128:(sc + 1) * 128],
                            rhs=wvT_sb, start=True, stop=True)
                    nc.vector.tensor_copy(
                        out=v_sb[:, half * 4:(half + 1) * 4, :],
                        in_=ps)
                VO = []
                for g in range(NG):
                    vo = qkv.tile([128, JC, 4, DH + 1], dt.bfloat16,
                                  tag=f"vo{g}", name=f"vo{g}")
                    nc.gpsimd.memset(vo[:, :, :, DH:DH + 1], 1.0)
                    nc.gpsimd.tensor_copy(
                        out=vo[:, :, :, 0:DH],
                        in_=v_sb[:, :, 64 * g:64 * g + 64].rearrange(
                            "p j (c d) -> p j c d", c=4))
                    VO.append(vo)
                QKV[b] = (qT, kT, v_sb, VO)

            for b in range(NB):
                qT, kT, v_sb, VO = QKV[b]
                oatt[b] = [att_p.tile([128, H, DH], dt.bfloat16,
                                      tag=f"oatt{sc}", name=f"oatt{sc}")
                           for sc in range(SC)]
                # ---------- attention ----------
                for ih in range(2):
                    for g in range(NG):
                        i0 = ih * 512
                        pv = accp.tile([128, 512], dt.float32, tag="pv",
                                       name="pv")
                        bvt = accp.tile([128, 512], dt.float32, tag="bv",
                                        name="bv")
                        for jc in range(JC):
                            uts = []
                            for pair in range(2):
                                st = scp.tile([128, 2, 512], dt.float32,
                                              tag="sc", name=f"sc{pair}")
                                for ci in range(2):
                                    c = 2 * pair + ci
                                    nc.tensor.matmul(
                                        st[:, ci, :],
                                        lhsT=kT[g][32 * c:32 * c + DH,
                                                   jc * 128:(jc + 1) * 128],
                                        rhs=qT[g][32 * c:32 * c + DH,
                                                  i0:i0 + 512],
                                        start=True, stop=True,
                                        tile_position=(32 * c, 0))
                                ut = sm.tile([128, 2, 512], dt.bfloat16,
                                             tag="ut", name=f"ut{pair}",
                                             bufs=4)
                                nc.scalar.activation(
                                    ut.rearrange("p a b -> p (a b)"),
                                    st.rearrange("p a b -> p (a b)"),
                                    AF.Exp, scale=SCALE)
                                uts.append(ut)
                            for c in range(4):
                                nc.tensor.matmul(
                                    pv[32 * c:32 * c + DH + 1, :],
                                    lhsT=VO[g][:, jc, c, :],
                                    rhs=uts[c // 2][:, c % 2, :],
                                    start=(jc == 0), stop=(jc == JC - 1),
                                    skip_group_check=True,
                                    tile_position=(0, 32 * c))
                            for c in range(4):
                                nc.tensor.matmul(
                                    bvt[32 * c:32 * c + DH, :],
                                    lhsT=v_sb[:, jc,
                                              64 * g + 16 * c:
                                              64 * g + 16 * c + DH],
                                    rhs=ts_sb[:, 4 * g + c,
                                              896 - 128 * jc + i0:
                                              896 - 128 * jc + i0 + 512],
                                    start=(jc == 0), stop=(jc == JC - 1),
                                    skip_group_check=True,
                                    tile_position=(0, 32 * c))
                        # ---- softmax tail for this (g, ih) ----
                        osb = sm.tile([128, 512], dt.float32, tag="osb",
                                      name="osb", bufs=3)
                        nc.vector.tensor_copy(out=osb, in_=pv)
                        bvc = sm.tile([128, 512], dt.float32, tag="bvc",
                                      name="bvc", bufs=3)
                        nc.vector.tensor_copy(out=bvc, in_=bvt)
                        rs = sm.tile([4, 512], dt.float32, tag="rs",
                                     name="rs")
                        # one strided DMA grabs all 4 denominator rows
                        nc.sync.dma_start(
                            out=rs, in_=osb[DH:DH + 97:32, :])
                        rr = sm.tile([4, 512], dt.float32, tag="rr",
                                     name="rr")
                        nc.vector.reciprocal(out=rr, in_=rs)
                        rsb = msp.tile([128, 512], dt.float32, tag="ms",
                                       name="rsb")
                        nc.tensor.matmul(rsb, lhsT=bcast4_sb, rhs=rr,
                                         start=True, stop=True)
                        nc.vector.tensor_tensor(osb, osb, rsb, op=ALU.mult)
                        osb2 = sm.tile([128, 512], dt.bfloat16, tag="osb2",
                                       name="osb2")
                        nc.vector.tensor_tensor(osb2, osb, bvc, op=ALU.add)
                        ot = sm.tile([128, 4, 128], dt.bfloat16, tag="ot",
                                     name="ot")
                        nc.sync.dma_start_transpose(out=ot, in_=osb2)
                        for ic in range(4):
                            sc = ih * 4 + ic
                            otr = ot[:, ic, :].rearrange("p (c m) -> p c m",
                                                         c=4)
                            nc.gpsimd.tensor_copy(
                                out=oatt[b][sc][:, 4 * g:4 * g + 4, :],
                                in_=otr[:, :, 0:DH])

            # ---------- LN / FFN / GAP ----------
            # Processed in two independent s-halves (sc 0-3, sc 4-7) so the
            # first half pipelines under the tail of the attention phase.
            def eng(j):
                return nc.gpsimd if j % 2 == 0 else nc.vector

            for b in range(NB):
                def ln_stats(tiles_in, tag):
                    """LN stats for 8 s-chunks: (mv [128,8,2], rstd [128,8]).

                    rstd = 1/sqrt(var+eps) via fast-inverse-sqrt bits +
                    one Newton step on DVE (rel err ~2e-3), avoiding ACT
                    table swaps against the attention exps.
                    """
                    mv = sm.tile([128, SC, 2], dt.float32, tag=f"mv{tag}",
                                 name=f"mv{tag}")
                    for sc in range(SC):
                        stats = sm.tile([128, 6], dt.float32, tag="stats",
                                        name="stats", bufs=6)
                        nc.vector.bn_stats(out=stats, in_=tiles_in(sc))
                        nc.vector.bn_aggr(out=mv[:, sc, :], in_=stats)
                    ve = sm.tile([128, SC], dt.float32, tag=f"ve{tag}",
                                 name=f"ve{tag}")
                    nc.vector.tensor_scalar(ve, mv[:, :, 1], EPS, None,
                                            ALU.add)
                    yi = sm.tile([128, SC], dt.int32, tag=f"yi{tag}",
                                 name=f"yi{tag}")
                    nc.vector.tensor_scalar(yi, ve.bitcast(dt.int32), 1, None,
                                            ALU.logical_shift_right)
                    nc.vector.tensor_scalar(yi, yi, -1, 0x5F3759DF,
                                            ALU.mult, ALU.add)
                    rstd = yi.bitcast(dt.float32)
                    tn = sm.tile([128, SC], dt.float32, tag=f"tn{tag}",
                                 name=f"tn{tag}")
                    nc.vector.tensor_tensor(tn, rstd, rstd, op=ALU.mult)
                    nc.vector.tensor_tensor(tn, tn, ve, op=ALU.mult)
                    nc.vector.tensor_scalar(tn, tn, -0.5, 1.5,
                                            ALU.mult, ALU.add)
                    nc.vector.tensor_tensor(rstd, rstd, tn, op=ALU.mult)
                    return mv, rstd

                def ln_apply(dst, src_ap, mv, rstd, sc, k):
                    eng(sc).tensor_scalar(dst, src_ap, mv[:, sc, 0:1],
                                          rstd[:, sc:sc + 1],
                                          ALU.subtract, ALU.mult)
                    if not ln_identity:
                        eng(sc).tensor_tensor(dst, dst, lnG_sb[:, k, :],
                                              op=ALU.mult)
                        eng(sc).tensor_tensor(dst, dst, lnB_sb[:, k, :],
                                              op=ALU.add)

                # LN(attn out) then residual-add then LN1
                mvA, rstdA = ln_stats(
                    lambda sc: oatt[b][sc].rearrange("p h d -> p (h d)"), "A")
                o1r = [sm.tile([128, 128], dt.bfloat16, tag=f"o1_{sc}",
                               name=f"o1_{sc}") for sc in range(SC)]
                for sc in range(SC):
                    ln_apply(o1r[sc], oatt[b][sc].rearrange("p h d -> p (h d)"),
                             mvA, rstdA, sc, 0)
                    eng(sc).tensor_tensor(o1r[sc], o1r[sc],
                                          xsrc[b][:, sc, :], op=ALU.add)
                mvB, rstdB = ln_stats(lambda sc: o1r[sc], "B")
                attB = att_p.tile([128, SC, 128], dt.bfloat16, tag="attB",
                                  name="attB")
                for sc in range(SC):
                    ln_apply(attB[:, sc, :], o1r[sc], mvB, rstdB, sc, 1)
                attT = att_p.tile([128, SC, 128], dt.bfloat16, tag="attT",
                                  name="attT")
                nc.sync.dma_start_transpose(
                    out=attT, in_=attB.rearrange("p a b -> p (a b)"))
                # FFN (relu + bias adds on ACT: idle during this phase)
                attTf = attT.rearrange("p a b -> p (a b)")
                hrelu = att_p.tile([128, 4, S], dt.bfloat16, tag="hr",
                                   name="hr")
                for fc in range(4):
                    for sh in range(2):
                        ps = msp.tile([128, 512], dt.float32, tag="ms",
                                      name="msf1")
                        nc.tensor.matmul(
                            ps, lhsT=ffw1T_sb[:, fc * 128:(fc + 1) * 128],
                            rhs=attTf[:, sh * 512:(sh + 1) * 512],
                            start=True, stop=True)
                        nc.scalar.activation(
                            hrelu[:, fc, sh * 512:(sh + 1) * 512], ps,
                            AF.Relu, bias=ffb1_sb[:, fc:fc + 1])
                ffT = att_p.tile([128, S], dt.bfloat16, tag="ffT", name="ffT")
                for sh in range(2):
                    ps = msp.tile([128, 512], dt.float32, tag="ms",
                                  name="msf2")
                    for fc in range(4):
                        nc.tensor.matmul(
                            ps, lhsT=ffw2T_sb[:, fc, :],
                            rhs=hrelu[:, fc, sh * 512:(sh + 1) * 512],
                            start=(fc == 0), stop=(fc == 3))
                    nc.scalar.activation(
                        ffT[:, sh * 512:(sh + 1) * 512], ps,
                        AF.Identity, bias=ffb2_sb[:, 0:1])
                ffTt = att_p.tile([128, SC, 128], dt.bfloat16, tag="ffTt",
                                  name="ffTt")
                nc.sync.dma_start_transpose(out=ffTt, in_=ffT)
                # residual 2 + LN2 + GAP
                l2r = [sm.tile([128, 128], dt.bfloat16, tag=f"l2_{sc}",
                               name=f"l2_{sc}") for sc in range(SC)]
                for sc in range(SC):
                    eng(sc).tensor_tensor(l2r[sc], attB[:, sc, :],
                                          ffTt[:, sc, :], op=ALU.add)
                mvC, rstdC = ln_stats(lambda sc: l2r[sc], "C")
                acc = sm.tile([128, 1], dt.float32, tag="acc", name="acc")
                nc.vector.memset(acc, 0.0)
                for sc in range(SC):
                    l2o = sm.tile([128, 128], dt.bfloat16, tag="l2o",
                                  name="l2o", bufs=4)
                    ln_apply(l2o, l2r[sc], mvC, rstdC, sc, 2)
                    mps = msp.tile([128, 1], dt.float32, tag="ms", name="msm")
                    nc.tensor.matmul(mps, lhsT=l2o, rhs=onesB,
                                     start=True, stop=True)
                    nc.vector.tensor_tensor(acc, acc, mps, op=ALU.add)
                ob = sm.tile([128, 1], dt.float32, tag="ob", name="ob")
                nc.vector.tensor_scalar(ob, acc, 1.0 / S, None, ALU.mult)
                nc.sync.dma_start(out=yout[b, :, None], in_=ob)

    nc.compile()
    return nc


_CACHE = {}


def _build(inputs):
    host, i16, i32, ln_identity = _host_prep(inputs)
    key = (ln_identity, host["cpk16"].shape[1], host["cpk32"].shape[1])
    if key not in _CACHE:
        _CACHE[key] = _build_bass(i16, i32, host["cpk16"].shape[1],
                                  host["cpk32"].shape[1], ln_identity)
    return _CACHE[key], host


def _make_in_maps(inputs, host):
    x = np.asarray(inputs["x"], dtype=F32)                 # [B, S, 4]
    xpad = np.zeros((B, S + 7, C_IN), F32)
    xpad[:, 3:S + 3, :] = x
    # rhs8x[b, 32*cc + t, r*S + s] = xpad[b, s + t, r]  (4 copies over cc)
    r8 = np.empty((B, 8, C_IN, S), F32)
    for t in range(8):
        r8[:, t] = xpad[:, t:t + S, :].transpose(0, 2, 1)
    r8 = r8.reshape(B, 8, C_IN * S).astype(BF16)
    rhs8x = np.zeros((B, 128, C_IN * S), BF16)
    for cc in range(4):
        rhs8x[:, 32 * cc:32 * cc + 8, :] = r8
    in_maps = []
    for core in range(N_CORES):
        m = {"rhs8x": np.ascontiguousarray(rhs8x[core * NB:(core + 1) * NB])}
        m.update(host)
        in_maps.append(m)
    return in_maps


def kernel(**inputs):
    inputs = {k: np.asarray(v) for k, v in inputs.items()}
    nc, host = _build(inputs)
    from concourse.bass_utils import run_bass_kernel_spmd
    in_maps = _make_in_maps(inputs, host)
    res = run_bass_kernel_spmd(nc, in_maps, list(range(N_CORES)))
    outs = [res.results[c]["y"] for c in range(N_CORES)]
    return np.concatenate(outs, axis=0).astype(F32)


def build(inputs):
    inputs = {k: np.asarray(v) for k, v in inputs.items()}
    nc, host = _build(inputs)
    return nc, _make_in_maps(inputs, host)



# revision 2
# speedup vs baseline: 14.3561x; 14.3561x over previous
"""Trainium2 Bass kernel for nn_ConvTran (conv stem + eRPE transformer + GAP).

Sharding: pure data parallel. B=16 split as 2 samples per core across 8 cores.
All parameters replicated; per-core outputs concatenated on host.

v3 (from v2 baseline):
- attention exps split ACT/DVE: heads {0,1} exact exp on ACT, heads {2,3}
  via the bf16 Schraudolph bit-trick on DVE (one tensor_scalar, ~3% exp
  err -> ~1e-3 end-to-end after softmax normalization)
- bvt shares the VO stationary operand with pv (halves attention ldweights)
- fused softmax tail: dropped the bvc copy (TT reads bvt PSUM directly);
  rsb broadcast reuses an "st" PSUM slot
- per-sample phase emission (qkv/attn/tail) so sample 0's LN/FFN tail
  overlaps sample 1's attention
- FFN relu+bias and ffn2 bias on DVE; residuals/LN-apply on GPSIMD
- GAP accumulated in PSUM (start/stop) instead of 8 DVE adds
"""

import numpy as np
import ml_dtypes

# ---- problem constants (hardcoded; kernel.py must be self-contained) ----
B, S, C_IN, E, H, DFF = 16, 1024, 4, 128, 8, 512
C1 = E * 4          # 512
DH = E // H         # 16
EPS = 1e-5
SCALE = float(E) ** -0.5
N_CORES = 8
NB = B // N_CORES   # samples per core = 2
NG = 2              # head groups of 4
SC = S // 128       # 8 s-chunks
JC = S // 128       # 8 j-chunks
F32 = np.float32
BF16 = ml_dtypes.bfloat16

# bf16 Schraudolph exp: i16 = round(s*TRICK_A + TRICK_B); bitcast bf16
TRICK_A = SCALE * (128.0 / np.log(2.0))      # folds the softmax scale
TRICK_B = 16256.0 - 5.6


class _Pack:
    """Column-packed [128, N] constant store."""

    def __init__(self, dtype):
        self.dtype = dtype
        self.cols = []
        self.index = {}
        self.n = 0

    def add(self, name, arr2d):
        a = np.zeros((128, arr2d.shape[1]), self.dtype)
        a[:arr2d.shape[0]] = arr2d.astype(self.dtype)
        self.index[name] = (self.n, arr2d.shape[1])
        self.cols.append(a)
        self.n += arr2d.shape[1]

    def finalize(self):
        return np.ascontiguousarray(np.concatenate(self.cols, axis=1))


def _host_prep(inp):
    f = lambda a: np.asarray(a, dtype=F32)
    pb = _Pack(BF16)   # matmul operands
    pf = _Pack(F32)    # activation scale/bias scalars

    # conv1 weights, 4 copies at 32-row offsets for row-tiled matmuls:
    # w1c4[32*cc + t, j] = conv1_w[cc*128 + j, 0, 0, t]
    w1 = f(inp["conv1_w"])[:, 0, 0, :]                 # [C1, 8]
    w1c4 = np.zeros((128, 128), F32)
    for cc in range(4):
        w1c4[32 * cc:32 * cc + 8, :] = w1[cc * 128:(cc + 1) * 128, :].T
    pb.add("w1c4", w1c4)
    sA = f(inp["bn1_g"]) / np.sqrt(f(inp["bn1_v"]) + EPS)
    pf.add("scaleA", sA.reshape(4, 128).T)
    pf.add("biasA", ((f(inp["conv1_b"]) - f(inp["bn1_m"])) * sA
                     + f(inp["bn1_b"])).reshape(4, 128).T)

    # conv2: [128(c1), 16(k=r*4+cc), 128(e)]
    w2 = f(inp["conv2_w"])[:, :, :, 0]                 # [E, C1, 4]
    w2cT = np.zeros((128, 16, 128), F32)
    for r in range(4):
        for cc in range(4):
            w2cT[:, r * 4 + cc, :] = w2[:, cc * 128:(cc + 1) * 128, r].T
    pb.add("w2cT", w2cT.reshape(128, 16 * 128))
    sB = f(inp["bn2_g"]) / np.sqrt(f(inp["bn2_v"]) + EPS)
    pf.add("scaleB", sB[:, None])
    pf.add("biasB", ((f(inp["conv2_b"]) - f(inp["bn2_m"])) * sB
                     + f(inp["bn2_b"]))[:, None])

    # tAPE fixed positional encoding, transposed [E, S]
    pos = np.arange(S, dtype=np.float64)[:, None]
    div = np.exp(np.arange(0, E, 2, dtype=np.float64) * (-np.log(10000.0) / E))
    ang = pos * div * (E / S)
    pe = np.zeros((S, E), np.float64)
    pe[:, 0::2] = np.sin(ang)
    pe[:, 1::2] = np.cos(ang)
    pb.add("peT", pe.T)

    # q/k weights, padded head layout [128, g*128 + 32c + dh]
    def pad_qk(w):
        w = f(w)
        wt = np.zeros((128, NG * 128), F32)
        for g in range(NG):
            for c in range(4):
                h = 4 * g + c
                wt[:, g * 128 + 32 * c:g * 128 + 32 * c + DH] = \
                    w[h * DH:(h + 1) * DH, :].T
        return wt
    pb.add("wqT", pad_qk(inp["wq"]))
    pb.add("wkT", pad_qk(inp["wk"]))
    pb.add("wvT", f(inp["wv"]).T)

    pb.add("ffw1T", f(inp["ff_w1"]).T)
    pf.add("ffb1", f(inp["ff_b1"]).reshape(4, 128).T)
    pb.add("ffw2T", f(inp["ff_w2"]).T.reshape(4, 128, 128)
           .transpose(1, 0, 2).reshape(128, 512))
    pf.add("ffb2", f(inp["ff_b2"])[:, None])

    m = np.arange(128)
    pf.add("bcast4", (m[None, :] // 32 == np.arange(4)[:, None]).astype(F32))
    pf.add("eps", np.full((128, 1), EPS, F32))
    pb.add("identB", np.eye(128, dtype=F32))
    pb.add("onesB", np.ones((128, 1), F32))

    lnG = np.stack([f(inp["ln_attn_g"]), f(inp["ln1_g"]), f(inp["ln2_g"])])
    lnB = np.stack([f(inp["ln_attn_b"]), f(inp["ln1_b"]), f(inp["ln2_b"])])
    ln_identity = bool(np.allclose(lnG, 1.0) and np.allclose(lnB, 0.0))
    pf.add("lnG", np.broadcast_to(lnG.reshape(1, 3 * 128), (128, 384)).copy())
    pf.add("lnB", np.broadcast_to(lnB.reshape(1, 3 * 128), (128, 384)).copy())

    d = {"cpk16": pb.finalize(), "cpk32": pf.finalize()}

    # rel_bias diagonal store (bf16): T[jj, h, c] = rel[127 + c - jj, h]
    rel = f(inp["rel_bias"])                          # [2047, 8]
    jj = np.arange(128)[:, None]
    cidx = np.arange(1920)[None, :]
    ts = rel[127 + cidx - jj, :]                      # [128, 1920, 8]
    d["tstore"] = np.ascontiguousarray(
        ts.transpose(0, 2, 1).astype(BF16))           # [128, 8, 1920]
    return d, pb.index, pf.index, ln_identity


def _build_bass(i16, i32, n16, n32, ln_identity, loop_r=1, sim_safe=False):
    import concourse.bass as bass
    import concourse.bacc as bacc
    import concourse.tile as tile
    import concourse.mybir as mybir

    dt = mybir.dt
    AF = mybir.ActivationFunctionType
    ALU = mybir.AluOpType

    nc = bacc.Bacc("TRN2")

    xin = nc.dram_tensor("rhs8x", [NB, 128, 4 * S], dt.bfloat16,
                         kind="ExternalInput")
    c16_dr = nc.dram_tensor("cpk16", [128, n16], dt.bfloat16,
                            kind="ExternalInput")
    c32_dr = nc.dram_tensor("cpk32", [128, n32], dt.float32,
                            kind="ExternalInput")
    ts_dr = nc.dram_tensor("tstore", [128, H, 1920], dt.bfloat16,
                           kind="ExternalInput")
    yout = nc.dram_tensor("y", [NB, E], dt.float32, kind="ExternalOutput")

    xb = 1 if loop_r == 1 else 2   # extra slack for For_i back-edge

    with tile.TileContext(nc) as tc:
        import contextlib
        ctx = contextlib.ExitStack()
        with ctx:
            consts = ctx.enter_context(tc.tile_pool(name="consts", bufs=1))
            c16 = consts.tile([128, n16], dt.bfloat16, tag="cpk16")
            nc.sync.dma_start(out=c16, in_=c16_dr[:])
            c32 = consts.tile([128, n32], dt.float32, tag="cpk32")
            nc.sync.dma_start(out=c32, in_=c32_dr[:])
            ts_sb = consts.tile([128, H, 1920], dt.bfloat16, tag="tstore")
            nc.sync.dma_start(out=ts_sb, in_=ts_dr[:])

            def C16(name, rows=128):
                o, w = i16[name]
                return c16[0:rows, o:o + w]

            def C32(name, rows=128):
                o, w = i32[name]
                return c32[0:rows, o:o + w]

            w1c4_sb = C16("w1c4")
            scaleA_sb, biasA_sb = C32("scaleA"), C32("biasA")
            w2cT_sb = C16("w2cT").rearrange("p (k e) -> p k e", k=16)
            scaleB_sb, biasB_sb = C32("scaleB"), C32("biasB")
            peT_sb = C16("peT")
            wqT_sb = C16("wqT").rearrange("p (g e) -> p g e", g=NG)
            wkT_sb = C16("wkT").rearrange("p (g e) -> p g e", g=NG)
            wvT_sb = C16("wvT")
            ffw1T_sb = C16("ffw1T")
            ffb1_sb = C32("ffb1")
            ffw2T_sb = C16("ffw2T").rearrange("p (k e) -> p k e", k=4)
            ffb2_sb = C32("ffb2")
            bcast4_sb = C32("bcast4", rows=4)
            identB = C16("identB")
            onesB = C16("onesB")
            lnG_sb = C32("lnG").rearrange("p (k e) -> p k e", k=3)
            lnB_sb = C32("lnB").rearrange("p (k e) -> p k e", k=3)

            loop_cm = tc.For_i(0, loop_r, 1) if loop_r > 1 else \
                contextlib.nullcontext()
            with loop_cm:
                _emit_body(nc, tc, ctx, dt, AF, ALU, xin, yout, ln_identity,
                           xb, w1c4_sb, scaleA_sb, biasA_sb, w2cT_sb,
                           scaleB_sb, biasB_sb, peT_sb, wqT_sb, wkT_sb,
                           wvT_sb, ffw1T_sb, ffb1_sb, ffw2T_sb, ffb2_sb,
                           bcast4_sb, identB, onesB, lnG_sb, lnB_sb, ts_sb,
                           sim_safe=sim_safe)

    nc.compile()
    return nc


def _emit_body(nc, tc, ctx, dt, AF, ALU, xin, yout, ln_identity, xb,
               w1c4_sb, scaleA_sb, biasA_sb, w2cT_sb, scaleB_sb, biasB_sb,
               peT_sb, wqT_sb, wkT_sb, wvT_sb, ffw1T_sb, ffb1_sb, ffw2T_sb,
               ffb2_sb, bcast4_sb, identB, onesB, lnG_sb, lnB_sb, ts_sb,
               sim_safe=False):
    import concourse.bass as bass

    pers = ctx.enter_context(tc.tile_pool(name="pers", bufs=1))
    xpT = [pers.tile([128, S], dt.bfloat16, tag=f"xpT{b}",
                     name=f"xpT{b}") for b in range(NB)]
    xsrc = [pers.tile([128, SC, 128], dt.bfloat16, tag=f"xsrc{b}",
                      name=f"xsrc{b}") for b in range(NB)]

    # =========== PHASE C: conv stem (both samples) ===========
    with tc.tile_pool(name="h1", bufs=1) as h1p, \
         tc.tile_pool(name="cps", bufs=1, space="PSUM") as cps, \
         tc.tile_pool(name="cmisc", bufs=2 + xb) as cmisc:
        for b in range(NB):
            rhs8 = cmisc.tile([128, 4 * S], dt.bfloat16, tag="rhs8",
                              name="rhs8")
            # split the load so conv1 r=0 starts on the first chunk
            for rq in range(4):
                nc.sync.dma_start(
                    out=rhs8[:, rq * S:(rq + 1) * S],
                    in_=xin[b][:, rq * S:(rq + 1) * S])
            h1 = [[h1p.tile([128, S], dt.bfloat16,
                            tag=f"h1_{cc}_{r}", name=f"h1_{cc}_{r}")
                   for r in range(4)] for cc in range(4)]
            # conv1 + BN1 + gelu, row-tiled over cc
            for r in range(4):
                for cc in range(4):
                    ps = cps.tile([128, 2, 512], dt.float32,
                                  tag="c1ps", name="c1ps", bufs=3)
                    for sh in range(2):
                        n = r * 2 + sh
                        nc.tensor.matmul(
                            ps[:, sh, :],
                            lhsT=w1c4_sb[32 * cc:32 * cc + 8, :],
                            rhs=rhs8[32 * cc:32 * cc + 8,
                                     n * 512:(n + 1) * 512],
                            start=True, stop=True,
                            tile_position=(32 * cc, 0))
                    nc.scalar.activation(
                        h1[cc][r], ps.rearrange("p a b -> p (a b)"),
                        AF.Gelu, bias=biasA_sb[:, cc:cc + 1],
                        scale=scaleA_sb[:, cc:cc + 1])
            # conv2 + BN2 + gelu
            xsT = cmisc.tile([128, S], dt.bfloat16, tag="xsT",
                             name="xsT")
            ps2 = cps.tile([128, 2, 512], dt.float32, tag="c2ps",
                           name="c2ps", bufs=1)
            for sh in range(2):
                for k in range(16):
                    r, cc = k // 4, k % 4
                    nc.tensor.matmul(
                        ps2[:, sh, :], lhsT=w2cT_sb[:, k, :],
                        rhs=h1[cc][r][:, sh * 512:(sh + 1) * 512],
                        start=(k == 0), stop=(k == 15))
            nc.scalar.activation(
                xsT, ps2.rearrange("p a b -> p (a b)"), AF.Gelu,
                bias=biasB_sb, scale=scaleB_sb)
            nc.gpsimd.tensor_tensor(xpT[b], xsT, peT_sb, op=ALU.add)
            nc.sync.dma_start_transpose(out=xsrc[b], in_=xsT)

    # =========== main pools: qkv + attention + tail ===========
    mp = ctx.enter_context(tc.tile_pool(name="mp", bufs=1, space="PSUM"))
    qkv = ctx.enter_context(tc.tile_pool(name="qkv", bufs=2))
    att_p = ctx.enter_context(tc.tile_pool(name="attp", bufs=2))
    sm = ctx.enter_context(tc.tile_pool(name="sm", bufs=3))

    def st_tile():
        return mp.tile([128, 2, 512], dt.float32, tag="st", name="st",
                       bufs=3)

    def pv_tile(tag):
        return mp.tile([128, 512], dt.float32, tag=tag, name=tag,
                       bufs=2)

    def qkv_attention(b):
        """QKV + rel-bias bursts + exp-gated attention for one sample.

        Group softmax tails are DEFERRED into the next group's step
        stream so the rsb broadcast matmul (which waits on the
        denominator DMA-gather chain) never head-of-line blocks PE.
        Returns the per-sample attention output tile."""
        qT, kT = [], []
        for g in range(NG):
            qt = qkv.tile([128, S], dt.bfloat16, tag=f"qt{g}",
                          name=f"qt{g}")
            kt = qkv.tile([128, S], dt.bfloat16, tag=f"kt{g}",
                          name=f"kt{g}")
            for dst, w in ((qt, wqT_sb), (kt, wkT_sb)):
                for sh in range(2):
                    ps = st_tile()
                    nc.tensor.matmul(ps[:, 0, :], lhsT=w[:, g, :],
                                     rhs=xpT[b][:, sh * 512:(sh + 1) * 512],
                                     start=True, stop=True)
                    nc.scalar.copy(
                        out=dst[:, sh * 512:(sh + 1) * 512],
                        in_=ps[:, 0, :])
            qT.append(qt)
            kT.append(kt)
        v_sb = qkv.tile([128, SC, 128], dt.bfloat16, tag="v", name="v")
        for half in range(2):
            ps = st_tile()
            for q4 in range(4):
                sc = half * 4 + q4
                nc.tensor.matmul(
                    ps[:, 0, 128 * q4:128 * q4 + 128],
                    lhsT=xpT[b][:, sc * 128:(sc + 1) * 128],
                    rhs=wvT_sb, start=True, stop=True)
            nc.vector.tensor_copy(
                out=v_sb[:, half * 4:(half + 1) * 4, :],
                in_=ps[:, 0, :])
        VO = []
        for g in range(NG):
            # 32-wide bands (v columns, ones column, zero pad) so pv/bvt
            # writes cover whole 32-row PSUM bands (no unwritten gaps)
            vo = qkv.tile([128, JC, 4, 32], dt.bfloat16,
                          tag=f"vo{g}", name=f"vo{g}")
            nc.gpsimd.memset(vo[:, :, :, DH:DH + 1], 1.0)
            nc.gpsimd.memset(vo[:, :, :, DH + 1:32], 0.0)
            nc.gpsimd.tensor_copy(
                out=vo[:, :, :, 0:DH],
                in_=v_sb[:, :, 64 * g:64 * g + 64].rearrange(
                    "p j (c d) -> p j c d", c=4))
            VO.append(vo)

        # rel-bias term: PE-dense burst phase (no exp dependency;
        # keeps pv free to double-buffer during attention)
        BVC = {}
        for ih in range(2):
            for g in range(NG):
                i0 = ih * 512
                bvt = pv_tile("pv")
                for jc in range(JC):
                    for c in range(4):
                        nc.tensor.matmul(
                            bvt[32 * c:32 * c + 32, :],
                            lhsT=VO[g][:, jc, c, :],
                            rhs=ts_sb[:, 4 * g + c,
                                      896 - 128 * jc + i0:
                                      896 - 128 * jc + i0 + 512],
                            start=(jc == 0), stop=(jc == JC - 1),
                            skip_group_check=True,
                            tile_position=(0, 32 * c))
                bvc = sm.tile([128, 512], dt.bfloat16, tag="bvc",
                              name="bvc", bufs=5)
                nc.vector.tensor_copy(out=bvc, in_=bvt)
                BVC[(ih, g)] = bvc

        oatt = att_p.tile([128, SC, H, DH], dt.bfloat16,
                          tag="oatt", name="oatt")

        def make_group_tail(osb, bvc, ih, g):
            def emit():
                rs = sm.tile([4, 512], dt.float32, tag="rs", name="rs")
                if sim_safe:
                    # CoreSim's interp mis-addresses partition-strided
                    # APs; use 4 contiguous row DMAs for validation
                    for c in range(4):
                        nc.sync.dma_start(
                            out=rs[c:c + 1, :],
                            in_=osb[32 * c + DH:32 * c + DH + 1, :])
                else:
                    # one strided DMA grabs all 4 denominator rows
                    nc.sync.dma_start(out=rs, in_=osb[DH:DH + 97:32, :])
                rr = sm.tile([4, 512], dt.float32, tag="rr", name="rr")
                nc.vector.reciprocal(out=rr, in_=rs)
                rsbt = pv_tile("pv")
                rsb = rsbt[:, 0:512]
                nc.tensor.matmul(rsb, lhsT=bcast4_sb, rhs=rr,
                                 start=True, stop=True)
                t1 = sm.tile([128, 512], dt.bfloat16, tag="t1",
                             name="t1", bufs=3)
                nc.vector.tensor_tensor(t1, osb, rsb, op=ALU.mult)
                ot = sm.tile([128, 4, 128], dt.bfloat16, tag="ot",
                             name="ot")
                nc.sync.dma_start_transpose(out=ot, in_=t1)
                btT = sm.tile([128, 4, 128], dt.bfloat16, tag="btT",
                              name="btT")
                nc.sync.dma_start_transpose(out=btT, in_=bvc)
                for ic in range(4):
                    sc = ih * 4 + ic
                    otr = ot[:, ic, :].rearrange("p (c m) -> p c m", c=4)
                    btr = btT[:, ic, :].rearrange("p (c m) -> p c m", c=4)
                    nc.gpsimd.tensor_tensor(
                        oatt[:, sc, 4 * g:4 * g + 4, :],
                        otr[:, :, 0:DH], btr[:, :, 0:DH], op=ALU.add)
            return emit

        deferred = []
        for ih in range(2):
            for g in range(NG):
                i0 = ih * 512
                pv = pv_tile("pv")
                pending = []

                def emit_pv(jc, uts, pv=pv, g=g):
                    for c in range(4):
                        nc.tensor.matmul(
                            pv[32 * c:32 * c + 32, :],
                            lhsT=VO[g][:, jc, c, :],
                            rhs=uts[c // 2][:, c % 2, :],
                            start=(jc == 0), stop=(jc == JC - 1),
                            skip_group_check=True,
                            tile_position=(0, 32 * c))

                for jc in range(JC):
                    if jc == 3 and deferred:
                        deferred.pop(0)()
                    uts = []
                    # heads 0,1 -> ACT exact exp
                    stA = st_tile()
                    for ci in range(2):
                        c = ci
                        nc.tensor.matmul(
                            stA[:, ci, :],
                            lhsT=kT[g][32 * c:32 * c + DH,
                                       jc * 128:(jc + 1) * 128],
                            rhs=qT[g][32 * c:32 * c + DH, i0:i0 + 512],
                            start=True, stop=True,
                            tile_position=(32 * c, 0))
                    utA = sm.tile([128, 2, 512], dt.bfloat16,
                                  tag="utA", name="utA", bufs=4)
                    nc.scalar.activation(
                        utA.rearrange("p a b -> p (a b)"),
                        stA.rearrange("p a b -> p (a b)"),
                        AF.Exp, scale=SCALE)
                    uts.append(utA)
                    # heads 2,3 -> DVE bit-trick exp
                    stB = st_tile()
                    for ci in range(2):
                        c = 2 + ci
                        nc.tensor.matmul(
                            stB[:, ci, :],
                            lhsT=kT[g][32 * c:32 * c + DH,
                                       jc * 128:(jc + 1) * 128],
                            rhs=qT[g][32 * c:32 * c + DH, i0:i0 + 512],
                            start=True, stop=True,
                            tile_position=(32 * c, 0))
                    utBi = sm.tile([128, 2, 512], dt.int16,
                                   tag="utB", name="utB", bufs=4)
                    nc.vector.tensor_scalar(
                        utBi.rearrange("p a b -> p (a b)"),
                        stB.rearrange("p a b -> p (a b)"),
                        TRICK_A, TRICK_B, ALU.mult, ALU.add)
                    uts.append(utBi.bitcast(dt.bfloat16))
                    # one-step software pipeline: pv for step jc-1 is
                    # emitted after step jc's scores, so PE's in-order
                    # queue never blocks on the current step's exps
                    pending.append((jc, uts))
                    if len(pending) > 1:
                        emit_pv(*pending.pop(0))
                for item in pending:
                    emit_pv(*item)
                # free pv promptly; the rest of the softmax tail is
                # deferred into the next group's step stream
                osb = sm.tile([128, 512], dt.float32, tag="osb",
                              name="osb", bufs=3)
                nc.scalar.copy(out=osb, in_=pv)
                deferred.append(make_group_tail(osb, BVC[(ih, g)], ih, g))
        for emit in deferred:
            emit()
        return oatt

    def ln_stats(tile_in, tag):
        """LN stats for 8 s-chunks: (mv [128,8,2], rstd [128,8]).

        rstd = 1/sqrt(var+eps) via fast-inverse-sqrt bits + one
        Newton step on DVE (rel err ~2e-3), avoiding ACT table swaps
        against the attention exps. Emitted in two chunks (yield).
        """
        mv = sm.tile([128, SC, 2], dt.float32, tag=f"mv{tag}",
                     name=f"mv{tag}")
        stats = sm.tile([128, SC, 6], dt.float32, tag="stats",
                        name="stats", bufs=3)
        for sc in range(SC):
            nc.vector.bn_stats(out=stats[:, sc, :],
                               in_=tile_in[:, sc, :])
        yield
        for sc in range(SC):
            nc.vector.bn_aggr(out=mv[:, sc, :], in_=stats[:, sc, :])
        ve = sm.tile([128, SC], dt.float32, tag=f"ve{tag}",
                     name=f"ve{tag}")
        nc.vector.tensor_scalar(ve, mv[:, :, 1], EPS, None, ALU.add)
        yi = sm.tile([128, SC], dt.int32, tag=f"yi{tag}",
                     name=f"yi{tag}")
        nc.vector.tensor_scalar(yi, ve.bitcast(dt.int32), 1, None,
                                ALU.logical_shift_right)
        nc.vector.tensor_scalar(yi, yi, -1, 0x5F3759DF,
                                ALU.mult, ALU.add)
        rstd = yi.bitcast(dt.float32)
        tn = sm.tile([128, SC], dt.float32, tag=f"tn{tag}",
                     name=f"tn{tag}")
        nc.vector.tensor_tensor(tn, rstd, rstd, op=ALU.mult)
        nc.vector.tensor_tensor(tn, tn, ve, op=ALU.mult)
        nc.vector.tensor_scalar(tn, tn, -0.5, 1.5, ALU.mult, ALU.add)
        nc.vector.tensor_tensor(rstd, rstd, tn, op=ALU.mult)
        yield (mv, rstd)

    def ln_apply(dst, src_ap, mv, rstd, sc, k):
        nc.gpsimd.tensor_scalar(dst, src_ap, mv[:, sc, 0:1],
                                rstd[:, sc:sc + 1],
                                ALU.subtract, ALU.mult)
        if not ln_identity:
            nc.gpsimd.tensor_tensor(dst, dst, lnG_sb[:, k, :],
                                    op=ALU.mult)
            nc.gpsimd.tensor_tensor(dst, dst, lnB_sb[:, k, :],
                                    op=ALU.add)

    def run_stats(gen):
        next(gen)
        return gen

    def tail_gen(b, oatt):
        """LN / FFN / GAP tail for one sample, chunked with yields so
        the two samples' chains interleave (latency hiding)."""
        oat_f = oatt.rearrange("p s h d -> p s (h d)")
        gA = ln_stats(oat_f, "A")
        next(gA)
        yield
        mvA, rstdA = next(gA)
        yield
        o1r = sm.tile([128, SC, 128], dt.bfloat16, tag="o1r", name="o1r")
        for sc in range(SC):
            ln_apply(o1r[:, sc, :], oat_f[:, sc, :], mvA, rstdA, sc, 0)
            nc.gpsimd.tensor_tensor(o1r[:, sc, :], o1r[:, sc, :],
                                    xsrc[b][:, sc, :], op=ALU.add)
        yield
        gB = ln_stats(o1r, "B")
        next(gB)
        yield
        mvB, rstdB = next(gB)
        yield
        attB = att_p.tile([128, SC, 128], dt.bfloat16, tag="attB",
                          name="attB")
        for sc in range(SC):
            ln_apply(attB[:, sc, :], o1r[:, sc, :], mvB, rstdB, sc, 1)
        attT = att_p.tile([128, SC, 128], dt.bfloat16, tag="attT",
                          name="attT")
        nc.sync.dma_start_transpose(
            out=attT, in_=attB.rearrange("p a b -> p (a b)"))
        yield
        # FFN: relu+bias and bias adds on DVE
        attTf = attT.rearrange("p a b -> p (a b)")
        hrelu = att_p.tile([128, 4, S], dt.bfloat16, tag="hr", name="hr")
        for fc in range(4):
            for sh in range(2):
                ps = st_tile()
                nc.tensor.matmul(
                    ps[:, 0, :],
                    lhsT=ffw1T_sb[:, fc * 128:(fc + 1) * 128],
                    rhs=attTf[:, sh * 512:(sh + 1) * 512],
                    start=True, stop=True)
                nc.vector.tensor_scalar(
                    hrelu[:, fc, sh * 512:(sh + 1) * 512], ps[:, 0, :],
                    ffb1_sb[:, fc:fc + 1], 0.0, ALU.add, ALU.max)
            if fc == 1:
                yield
        yield
        ffT = att_p.tile([128, S], dt.bfloat16, tag="ffT", name="ffT")
        for sh in range(2):
            ps = st_tile()
            for fc in range(4):
                nc.tensor.matmul(
                    ps[:, 0, :], lhsT=ffw2T_sb[:, fc, :],
                    rhs=hrelu[:, fc, sh * 512:(sh + 1) * 512],
                    start=(fc == 0), stop=(fc == 3))
            nc.vector.tensor_scalar(
                ffT[:, sh * 512:(sh + 1) * 512], ps[:, 0, :],
                ffb2_sb[:, 0:1], None, ALU.add)
        ffTt = att_p.tile([128, SC, 128], dt.bfloat16, tag="ffTt",
                          name="ffTt")
        nc.sync.dma_start_transpose(out=ffTt, in_=ffT)
        yield
        # residual 2 + LN2 + GAP
        l2r = sm.tile([128, SC, 128], dt.bfloat16, tag="l2r", name="l2r")
        for sc in range(SC):
            nc.gpsimd.tensor_tensor(l2r[:, sc, :], attB[:, sc, :],
                                    ffTt[:, sc, :], op=ALU.add)
        yield
        gC = ln_stats(l2r, "C")
        next(gC)
        yield
        mvC, rstdC = next(gC)
        yield
        mps = st_tile()
        for sc in range(SC):
            l2o = sm.tile([128, 128], dt.bfloat16, tag="l2o",
                          name="l2o", bufs=4)
            ln_apply(l2o, l2r[:, sc, :], mvC, rstdC, sc, 2)
            nc.tensor.matmul(mps[:, 0, 0:1], lhsT=l2o, rhs=onesB,
                             start=(sc == 0), stop=(sc == SC - 1))
        ob = sm.tile([128, 1], dt.float32, tag="ob", name="ob")
        nc.vector.tensor_scalar(ob, mps[:, 0, 0:1], 1.0 / S, None,
                                ALU.mult)
        nc.sync.dma_start(out=yout[b, :, None], in_=ob)

    oatts = [qkv_attention(b) for b in range(NB)]
    # merged tails: round-robin the two samples' chains so their
    # cross-engine latencies hide under each other
    gens = [tail_gen(b, oatts[b]) for b in range(NB)]
    while gens:
        nxt = []
        for gen in gens:
            try:
                next(gen)
                nxt.append(gen)
            except StopIteration:
                pass
        gens = nxt


_CACHE = {}


def _build(inputs, loop_r=1, sim_safe=False):
    host, i16, i32, ln_identity = _host_prep(inputs)
    key = (ln_identity, host["cpk16"].shape[1], host["cpk32"].shape[1],
           loop_r, sim_safe)
    if key not in _CACHE:
        _CACHE[key] = _build_bass(i16, i32, host["cpk16"].shape[1],
                                  host["cpk32"].shape[1], ln_identity,
                                  loop_r=loop_r, sim_safe=sim_safe)
    return _CACHE[key], host


def _make_in_maps(inputs, host):
    x = np.asarray(inputs["x"], dtype=F32)                 # [B, S, 4]
    xpad = np.zeros((B, S + 7, C_IN), F32)
    xpad[:, 3:S + 3, :] = x
    # rhs8x[b, 32*cc + t, r*S + s] = xpad[b, s + t, r]  (4 copies over cc)
    r8 = np.empty((B, 8, C_IN, S), F32)
    for t in range(8):
        r8[:, t] = xpad[:, t:t + S, :].transpose(0, 2, 1)
    r8 = r8.reshape(B, 8, C_IN * S).astype(BF16)
    rhs8x = np.zeros((B, 128, C_IN * S), BF16)
    for cc in range(4):
        rhs8x[:, 32 * cc:32 * cc + 8, :] = r8
    in_maps = []
    for core in range(N_CORES):
        m = {"rhs8x": np.ascontiguousarray(rhs8x[core * NB:(core + 1) * NB])}
        m.update(host)
        in_maps.append(m)
    return in_maps


def kernel(**inputs):
    inputs = {k: np.asarray(v) for k, v in inputs.items()}
    nc, host = _build(inputs)
    from concourse.bass_utils import run_bass_kernel_spmd
    in_maps = _make_in_maps(inputs, host)
    res = run_bass_kernel_spmd(nc, in_maps, list(range(N_CORES)))
    outs = [res.results[c]["y"] for c in range(N_CORES)]
    return np.concatenate(outs, axis=0).astype(F32)


def build(inputs, loop_r=1, sim_safe=False):
    inputs = {k: np.asarray(v) for k, v in inputs.items()}
    nc, host = _build(inputs, loop_r=loop_r, sim_safe=sim_safe)
    return nc, _make_in_maps(inputs, host)
